# revision 21
# baseline (speedup 1.0000x reference)
"""Trainium2 Bass kernel for nn_BL_36721970381090 (dense_mlp).

Math: the reference collapses to out[b] = M2 @ relu(M1 @ vec(x[b]) + b1) + b2,
M1 = perm(kron(W11, fc2_w)) [600, 400] (exact rank 200), M2 [3, 600].

Per 512-batch block (u-major intermediate g = 200 rows split 120+80):
  A (8 MMs): pg [128, 1024] psum (2 banks):
     bank0 = g0 (u in {0,1,2}) [120 r], bank1 = g1 (u in {3,4}) [80 r + 40 zero r]
     each bank accumulates 4 chunk-MMs contracting 100 x-rows.
  g-copy (1 op): pg[0:120, 0:1024] -> g_sb bf16. g_sb is a STATIC tile whose
     row 120 holds constant 1.0 (written once) -> bias1 is folded into stage B
     as contraction row 120 of the stationary.
  B (5 MMs): y chunk m = (t, u=m) -> y_super [128, 2560] psum (5 banks,
     chunk m in bank m); lhsT = bw2[0:121, 120m:...] (row 120 = bias1[:, m]).
  relu (1 op): y_super[0:120, 0:2560] -> r_super bf16 (no bias needed).
  C (5 MMs): col-tiled m=0..3 at tile_position (0, 32m) -> concurrent in one
     PE span; partials in pc bank partitions {0-2,32-34,64-66,96-98}; m=4
     accumulates onto the (0,0) slice (start=False).
  pc-copy (1 op) -> osb; 4 partial [3, *] slices summed on host.

Engine economy: 3 ACT/DVE ops per block (vs 12 naive) because engine cost is
free-size driven (~1ns/elem + ~130ns), partitions are free.
"""

import numpy as np
import ml_dtypes
from contextlib import ExitStack

import concourse.bass as bass
import concourse.bacc as bacc
import concourse.mybir as mybir
from concourse.bass import ds
from concourse.tile import TileContext
from concourse.bass_utils import run_bass_kernel_spmd

B, D1, D2 = 131072, 40, 10
T0, T1, O0 = 120, 5, 3
NCORES = 8
BC = B // NCORES          # 16384 batch per core
NB = 512                  # psum free-dim block
NBLK = BC // NB           # 32 blocks
XGRP = 1                  # blocks per x DMA
OGRP = 4                  # blocks per output DMA group

F32 = mybir.dt.float32
BF16 = mybir.dt.bfloat16
BF = ml_dtypes.bfloat16
RELU = mybir.ActivationFunctionType.Relu
COPY = mybir.ActivationFunctionType.Copy
ADD = mybir.AluOpType.add
MAX = mybir.AluOpType.max

COLTILE = True            # col-tiled stage C (4 concurrent MMs)

_CACHE = {}


def _build_nc():
    nc = bacc.Bacc()
    xt = nc.dram_tensor("xt", (101, NBLK * 4 * NB), BF16, kind="ExternalInput")
    aw = nc.dram_tensor("aw", (101, 1024), BF16, kind="ExternalInput")
    bw = nc.dram_tensor("bw", (121, 600), BF16, kind="ExternalInput")
    cw = nc.dram_tensor("cw", (120, 15), BF16, kind="ExternalInput")
    ones = nc.dram_tensor("ones", (1, 1024), BF16, kind="ExternalInput")
    outp = nc.dram_tensor("outp", (12, BC), F32, kind="ExternalOutput")

    with TileContext(nc) as tc, ExitStack() as ctx:
        consts = ctx.enter_context(tc.tile_pool(name="consts", bufs=1))
        aw_sb = consts.tile([101, 1024], BF16, tag="aw")
        bw_sb = consts.tile([121, 600], BF16, tag="bw")
        cw_sb = consts.tile([120, 15], BF16, tag="cw")

        xpool = ctx.enter_context(tc.tile_pool(name="xp", bufs=3))
        gspool = ctx.enter_context(tc.tile_pool(name="gs", bufs=2))
        rpool = ctx.enter_context(tc.tile_pool(name="rp", bufs=3))
        opool = ctx.enter_context(tc.tile_pool(name="op", bufs=2))
        pgpool = ctx.enter_context(tc.tile_pool(name="pg", bufs=1, space="PSUM"))
        pypool = ctx.enter_context(tc.tile_pool(name="py", bufs=1, space="PSUM"))
        pcpool = ctx.enter_context(tc.tile_pool(name="pc", bufs=1, space="PSUM"))

        xts = {}

        def issue_x_dma(jj):
            if jj % XGRP == 0 and jj < NBLK:
                xt2 = xpool.tile([101, XGRP * 4 * NB], BF16, tag="xt2",
                                 name=f"xt2_{jj}")
                nc.sync.dma_start(xt2[:, :],
                                  xt[:, ds(jj * 4 * NB, XGRP * 4 * NB)])
                xts[jj // XGRP] = xt2

        # x DMA first (A(0) critical path), then the small consts
        issue_x_dma(0)
        nc.sync.dma_start(aw_sb[:, :], aw[:, :])
        issue_x_dma(1)
        nc.sync.dma_start(bw_sb[:, :], bw[:, :])
        nc.sync.dma_start(cw_sb[:, :], cw[:, :])

        def issue_A(jj):
            """DMA prefetch + 8 A-matmuls into a fresh pg tile. Returns pg."""
            issue_x_dma(jj + 2)
            xt2 = xts[jj // XGRP]
            xoff = (jj % XGRP) * 4 * NB
            pg = pgpool.tile([128, 2 * NB], F32, tag="pg", name=f"pg_{jj}")
            # g0 bank first so gcopy-a can fire mid-A
            for c in range(4):
                nc.tensor.matmul(pg[0:121, ds(0, NB)],
                                 aw_sb[:, ds(256 * c, 121)],
                                 xt2[:, ds(xoff + NB * c, NB)],
                                 start=(c == 0), stop=(c == 3))
            for c in range(4):
                nc.tensor.matmul(pg[0:121, ds(NB, NB)],
                                 aw_sb[:, ds(256 * c + 128, 121)],
                                 xt2[:, ds(xoff + NB * c, NB)],
                                 start=(c == 0), stop=(c == 3))
            return pg

        rtiles = {}
        state = {"osb": None}

        def issue_gcopy(pg, jj):
            # pooled g tiles (bufs=2) -> full cross-block pipelining;
            # row 120 = 1.0 ones (bias-fold) via idle gpsimd memset
            g0_sb = gspool.tile([128, NB], BF16, tag="g0sb", name=f"g0s_{jj}")
            g1_sb = gspool.tile([128, NB], BF16, tag="g1sb", name=f"g1s_{jj}")
            nc.scalar.activation(g0_sb[0:121, :], pg[0:121, ds(0, NB)], COPY)
            nc.vector.tensor_copy(g1_sb[0:121, :], pg[0:121, ds(NB, NB)])
            return g0_sb, g1_sb

        def issue_C(jc):
            """Stage C for block jc (relu outputs are a full cycle old)."""
            r_a, r_b, r_c = rtiles.pop(jc)

            def r_src(m):
                if m < 2:
                    return r_a[0:120, ds(NB * m, NB)]
                if m < 4:
                    return r_b[0:120, ds(NB * (m - 2), NB)]
                return r_c[0:120, :]

            pc = pcpool.tile([128, NB], F32, tag="pc", name=f"pc_{jc}")
            if COLTILE:
                for m in range(4):
                    nc.tensor.matmul(pc[32 * m:32 * m + 3, :],
                                     cw_sb[:, ds(3 * m, 3)], r_src(m),
                                     start=True, stop=(m != 0),
                                     tile_position=(0, 32 * m),
                                     skip_group_check=True)
                nc.tensor.matmul(pc[0:3, :], cw_sb[:, ds(12, 3)], r_src(4),
                                 start=False, stop=True, tile_position=(0, 0),
                                 skip_group_check=True)
            else:
                for m in range(5):
                    nc.tensor.matmul(pc[0:3, :], cw_sb[:, ds(3 * m, 3)],
                                     r_src(m), start=(m == 0), stop=(m == 4))
            # drain partials
            if jc % OGRP == 0:
                state["osb"] = opool.tile([128, OGRP * NB], F32, tag="osb",
                                          name=f"osb_{jc}")
            osb = state["osb"]
            oslot = (jc % OGRP) * NB
            rows = 99 if COLTILE else 3
            nc.vector.tensor_copy(osb[0:rows, ds(oslot, NB)], pc[0:rows, :])
            if jc % OGRP == OGRP - 1:
                gslice = ds((jc // OGRP) * OGRP * NB, OGRP * NB)
                if COLTILE:
                    for m in range(4):
                        nc.sync.dma_start(outp[ds(3 * m, 3), gslice],
                                          osb[32 * m:32 * m + 3, :])
                else:
                    nc.sync.dma_start(outp[ds(0, 3), gslice], osb[0:3, :])

        pg = issue_A(0)
        gsb01 = issue_gcopy(pg, 0)
        for jj in range(NBLK):
            g0_sb, g1_sb = gsb01
            # --- stage B: y_super [128, 2560] (chunk m in bank m) ---
            ys = pypool.tile([128, 5 * NB], F32, tag="ys", name=f"ys_{jj}")
            for m in range(5):
                gsb = g0_sb if m < 3 else g1_sb
                nc.tensor.matmul(ys[0:120, ds(NB * m, NB)],
                                 bw_sb[0:121, ds(120 * m, 120)],
                                 gsb[0:121, :],
                                 start=True, stop=True)
            # relu in 3 parallel ops: ACT chunks 0-1, DVE 2-3, ACT 4
            r_a = rpool.tile([128, 2 * NB], BF16, tag="ra", name=f"ra_{jj}")
            r_b = rpool.tile([128, 2 * NB], BF16, tag="rb", name=f"rb_{jj}")
            r_c = rpool.tile([128, NB], BF16, tag="rc", name=f"rc_{jj}")
            nc.scalar.activation(r_a[0:120, :], ys[0:120, ds(0, 2 * NB)], RELU)
            nc.vector.tensor_scalar(r_b[0:120, :], ys[0:120, ds(2 * NB, 2 * NB)],
                                    0.0, None, op0=MAX)
            nc.scalar.activation(r_c[0:120, :], ys[0:120, ds(4 * NB, NB)], RELU)
            rtiles[jj] = (r_a, r_b, r_c)

            # next block's A-matmuls + g-copies run while relu(jj) drains
            if jj + 1 < NBLK:
                pg = issue_A(jj + 1)
                gsb01 = issue_gcopy(pg, jj + 1)

            # stage C for the PREVIOUS block: its relus completed a full
            # cycle ago, so the C matmuls never stall the PE
            if jj >= 1:
                issue_C(jj - 1)
        issue_C(NBLK - 1)
    nc.finalize()
    return nc


def _host_mats(W11, fc2_w, W12, fc4_w, bias1):
    """Build aw [100,1024], bw [121,600], cw [120,15] (fp32)."""
    aw = np.zeros((101, 1024), np.float32)
    aw[100, 120] = 1.0        # c=0 g0-slab ones col (picks up xt ones row)
    aw[100, 128 + 120] = 1.0  # c=0 g1-slab ones col
    for c in range(4):
        for p in range(100):
            d = 10 * c + p // 10
            s = p % 10
            for u in range(3):
                aw[p, 256 * c + u * 40 + d] = fc2_w[u, s]
            for u in (3, 4):
                aw[p, 256 * c + 128 + (u - 3) * 40 + d] = fc2_w[u, s]
    bw = np.zeros((121, 600), np.float32)
    for m in range(5):
        if m < 3:
            rows = slice(40 * m, 40 * m + 40)
        else:
            rows = slice(40 * (m - 3), 40 * (m - 3) + 40)
        bw[rows, 120 * m:120 * m + 120] = W11.T        # [40 d, 120 t]
        bw[120, 120 * m:120 * m + 120] = bias1[:, m]   # folded bias1
    M2 = np.kron(W12, fc4_w)                            # [3, 600]
    cw = np.zeros((120, 15), np.float32)
    for m in range(5):
        cw[:, 3 * m:3 * m + 3] = M2[:, m::5].T
    return aw, bw, cw


def kernel(x, W11, fc2_w, bias1, W12, fc4_w, bias2, _trace=False):
    x = np.asarray(x, dtype=np.float32)
    W11 = np.asarray(W11, np.float32)
    fc2_w = np.asarray(fc2_w, np.float32)
    W12 = np.asarray(W12, np.float32)
    fc4_w = np.asarray(fc4_w, np.float32)
    b1m = np.asarray(bias1, np.float32)                 # [120, 5]
    b2v = np.asarray(bias2, np.float32)[:, 0]

    aw, bw, cw = _host_mats(W11, fc2_w, W12, fc4_w, b1m)
    awb = aw.astype(BF)
    bwb = bw.astype(BF)
    cwb = cw.astype(BF)
    onesb = np.ones((1, 1024), np.float32).astype(BF)

    if "nc" not in _CACHE:
        _CACHE["nc"] = _build_nc()
    nc = _CACHE["nc"]

    in_maps = []
    for c in range(NCORES):
        xs = x[c * BC:(c + 1) * BC]                      # [16384, 40, 10]
        xsr = xs.reshape(NBLK, NB, 400).transpose(2, 0, 1)
        xtc = np.empty((101, NBLK * 4 * NB), dtype=BF)
        xtc[0:100] = np.ascontiguousarray(
            xsr.reshape(4, 100, NBLK, NB).transpose(1, 2, 0, 3)
        ).reshape(100, NBLK * 4 * NB).astype(BF)
        xtc[100] = np.ones((NBLK * 4 * NB,), dtype=BF)
        in_maps.append({"xt": xtc, "aw": awb, "bw": bwb, "cw": cwb,
                        "ones": onesb})

    res = run_bass_kernel_spmd(nc, in_maps, core_ids=list(range(NCORES)),
                               trace=_trace)
    outs = []
    for c in range(NCORES):
        op = np.asarray(res.results[c]["outp"], np.float32)  # [12, BC]
        if COLTILE:
            outs.append(op.reshape(4, 3, BC).sum(axis=0))
        else:
            outs.append(op.reshape(4, 3, BC)[0])
    full = np.concatenate(outs, axis=1).T + b2v[None, :]
    if _trace:
        kernel.last_exec_time_ns = res.exec_time_ns
    return full.astype(np.float32)


# revision 22
# speedup vs baseline: 3.2553x; 3.2553x over previous
"""Trainium2 Bass kernel for nn_BL_36721970381090 (dense_mlp).

Math: the reference network
    item1 = einsum("td,bds->bts", W11, x)
    item2 = relu(einsum("bts,us->btu", item1, fc2_w) + bias1)
    item3 = einsum("ot,btu->bou", W12, item2)
    out   = (einsum("bou,pu->bop", item3, fc4_w) + bias2)[..., 0]
collapses (Kronecker identity) to a plain 2-layer MLP applied per batch row:
    out[b] = M2 @ relu(M1 @ vec(x[b]) + b1) + b2
with M1 = kron(W11, fc2_w) [600, 400], M2 = kron(W12, fc4_w) [3, 600],
b1 = bias1.reshape(600), b2 = bias2[:, 0].

Strategy: pure data parallel over 8 NeuronCores (batch split 131072 -> 8 x
16384). Host pre-transposes x to feature-major xT [400, Bc] per core and casts
to bf16 (input is the only large tensor; bf16 halves HBM traffic and doubles
PE throughput vs fp32's 2-pass matmul). On-chip: feature-major pipeline with
batch in the moving free dim - no on-chip transposes at all.
  layer1: psum[(t,u) chunk 120, b 512] += M1T_k[100,120].T @ xT_k[100,512]
          (4 K-chunks x 5 M-chunks)
  relu+bias1 on ScalarE (PSUM -> SBUF, cast to bf16)
  layer2: psum[3, b 512] += M2T_m[120,3].T @ relu_m[120,512]  (5 chunks)
  bias2 folded on host after gather.
"""

import numpy as np
import ml_dtypes
from contextlib import ExitStack

import concourse.bass as bass
import concourse.bacc as bacc
import concourse.mybir as mybir
from concourse.bass import ds
from concourse.tile import TileContext
from concourse.bass_utils import run_bass_kernel_spmd

B, D1, D2 = 131072, 40, 10
T0, T1, O0 = 120, 5, 3
NCORES = 8
BC = B // NCORES          # 16384 batch per core
KF = D1 * D2              # 400 input features (d, s)
MF = T0 * T1              # 600 hidden features (t, u)
KC = 100                  # K-chunk (4 chunks of 100 partitions)
MC = 120                  # M-chunk (5 chunks of 120 partitions)
NB = 512                  # matmul free-dim block (1 PSUM bank fp32)
NBD = 2048                # DMA block (4 x NB)

F32 = mybir.dt.float32
BF16 = mybir.dt.bfloat16
BF = ml_dtypes.bfloat16
RELU = mybir.ActivationFunctionType.Relu
COPY = mybir.ActivationFunctionType.Copy
ADD = mybir.AluOpType.add
MAX = mybir.AluOpType.max

_CACHE = {}


def _build_nc():
    nc = bacc.Bacc()
    xt = nc.dram_tensor("xt", (KF, BC), BF16, kind="ExternalInput")
    ablk = nc.dram_tensor("ablk", (KC, 256), BF16, kind="ExternalInput")
    l2a = nc.dram_tensor("l2a", (128, MF), BF16, kind="ExternalInput")
    l2b = nc.dram_tensor("l2b", (128, MF), BF16, kind="ExternalInput")
    m2t = nc.dram_tensor("m2t", (MC, 15), BF16, kind="ExternalInput")
    b1 = nc.dram_tensor("b1", (MC, 5), F32, kind="ExternalInput")
    outT = nc.dram_tensor("outT", (12, BC), F32, kind="ExternalOutput")

    nm = MF // MC  # 5

    with TileContext(nc) as tc, ExitStack() as ctx:
        consts = ctx.enter_context(tc.tile_pool(name="consts", bufs=1))
        a_sb = consts.tile([KC, 256], BF16, tag="a")
        nc.sync.dma_start(a_sb[:, :], ablk[:, :])
        l2_sb = [consts.tile([128, MF], BF16, tag=f"l2_{p}", name=f"l2sb{p}") for p in range(2)]
        nc.sync.dma_start(l2_sb[0][:, :], l2a[:, :])
        nc.sync.dma_start(l2_sb[1][:, :], l2b[:, :])
        m2_sb = consts.tile([MC, 15], BF16, tag="m2")
        nc.sync.dma_start(m2_sb[:, :], m2t[:, :])
        b1_sb = consts.tile([MC, nm], F32, tag="b1")
        nc.sync.dma_start(b1_sb[:, :], b1[:, :])
        xpool = ctx.enter_context(tc.tile_pool(name="xp", bufs=3))
        opool = ctx.enter_context(tc.tile_pool(name="op", bufs=2))
        zpool = ctx.enter_context(tc.tile_pool(name="zp", bufs=3))
        rpool = ctx.enter_context(tc.tile_pool(name="rp", bufs=6))
        pzp = ctx.enter_context(tc.tile_pool(name="pz", bufs=2, space="PSUM"))
        ps1p = ctx.enter_context(tc.tile_pool(name="ps1", bufs=4, space="PSUM"))
        ps2p = ctx.enter_context(tc.tile_pool(name="ps2", bufs=2, space="PSUM"))

        for blk in range(BC // NBD):
            if blk == 0:
                x0 = [
                    [xpool.tile([KC, NB], BF16, tag=f"w{k}_{jj}", name=f"x0_{k}_{jj}")
                     for k in range(4)]
                    for jj in range(NBD // NB)
                ]
                for jj in range(NBD // NB):
                    for k in range(4):
                        nc.sync.dma_start(
                            x0[jj][k][:, :],
                            xt[ds(k * KC, KC), ds(jj * NB, NB)],
                        )
            else:
                xk = [xpool.tile([KC, NBD], BF16, tag=f"x{k}", name=f"xk{k}") for k in range(4)]
                for k in range(4):
                    nc.sync.dma_start(xk[k][:, :], xt[ds(k * KC, KC), ds(blk * NBD, NBD)])
            for jj in range(NBD // NB):
                def xs(k):
                    return (x0[jj][k][:, :] if blk == 0
                            else xk[k][:, ds(jj * NB, NB)])
                # stage 1: z pair tiles [128, 512]; chunk 2p -> rows 0:64,
                # chunk 2p+1 -> rows 64:128 (zero-padded cols 50-63 in ablk)
                ztiles = []
                for p in range(2):
                    pz = pzp.tile([128, NB], F32, tag="pz", name=f"pz{p}{jj}")
                    nc.tensor.matmul(pz[0:128, :], a_sb[:, 0:128], xs(2 * p),
                                     start=True, stop=False)
                    nc.tensor.matmul(pz[0:128, :], a_sb[:, 128:256], xs(2 * p + 1),
                                     start=False, stop=True)
                    z = zpool.tile([128, NB], BF16, tag=f"z{p}", name=f"zt{p}{jj}")
                    if p == 0:
                        nc.scalar.activation(z[:, :], pz[:, :], COPY)
                    else:
                        nc.vector.tensor_copy(z[:, :], pz[:, :])
                    ztiles.append(z)
                # stage 2 + relu + layer 2
                rtiles = []
                for m in range(nm):
                    pp = ps1p.tile([MC, NB], F32, tag="ps1", name=f"pp{m}{jj}")
                    for p in range(2):
                        nc.tensor.matmul(
                            pp[:, :], l2_sb[p][:, ds(m * MC, MC)], ztiles[p][:, :],
                            start=(p == 0), stop=(p == 1),
                        )
                    r = rpool.tile([MC, NB], BF16, tag=f"r{m}", name=f"rt{m}{jj}")
                    if m < 3:
                        nc.scalar.activation(r[:, :], pp[:, :], RELU,
                                             bias=b1_sb[:, ds(m, 1)])
                    else:
                        nc.vector.tensor_scalar(r[:, :], pp[:, :],
                                                b1_sb[:, ds(m, 1)], 0.0,
                                                op0=ADD, op1=MAX)
                    rtiles.append(r)
                if jj == 0:
                    osb = opool.tile([128, NBD], F32, tag="osb")
                ps2 = ps2p.tile([128, NB], F32, tag="ps2", name=f"ps2{jj}")
                for m in range(4):
                    nc.tensor.matmul(
                        ps2[32 * m:32 * m + 3, :], m2_sb[:, ds(3 * m, 3)],
                        rtiles[m][:, :], start=True, stop=(m != 0),
                        tile_position=(0, 32 * m), skip_group_check=True,
                    )
                nc.tensor.matmul(
                    ps2[0:3, :], m2_sb[:, ds(12, 3)], rtiles[4][:, :],
                    start=False, stop=True, tile_position=(0, 0),
                    skip_group_check=True,
                )
                nc.vector.tensor_copy(osb[0:99, ds(jj * NB, NB)], ps2[0:99, :])
            for m in range(4):
                nc.sync.dma_start(outT[ds(3 * m, 3), ds(blk * NBD, NBD)],
                                  osb[32 * m:32 * m + 3, :])
    nc.finalize()
    return nc


def kernel(x, W11, fc2_w, bias1, W12, fc4_w, bias2, _trace=False):
    x = np.asarray(x, dtype=np.float32)
    W11 = np.asarray(W11, np.float32)
    fc2_w = np.asarray(fc2_w, np.float32)
    M2 = np.kron(np.asarray(W12, np.float32), np.asarray(fc4_w, np.float32))
    b1v = np.ascontiguousarray(np.asarray(bias1, np.float32).reshape(5, MC).T)
    b2v = np.asarray(bias2, np.float32)[:, 0]

    # stage-1 stationary: [(dl,s) 100, (u,dl') 50 + 14 zero pad]
    A50 = np.einsum("us,de->dsue", fc2_w,
                    np.eye(10, dtype=np.float32)).reshape(KC, 50)
    A = np.zeros((KC, 256), np.float32)
    A[:, 0:50] = A50
    A[:, 192:242] = A50
    ablk = np.ascontiguousarray(A).astype(BF)
    # stage-2 stationaries: pair p rows = chunk 2p (0:50), pad, chunk 2p+1 (64:114), pad
    l2 = np.zeros((2, 128, MF), np.float32)
    for p in range(2):
        for r in range(128):
            if r < 50:
                u, dl, d = r // 10, r % 10, 10 * (2 * p) + (r % 10)
            elif 64 <= r < 114:
                u, dl, d = (r - 64) // 10, (r - 64) % 10, 10 * (2 * p + 1) + ((r - 64) % 10)
            else:
                continue
            for t in range(T0):
                l2[p, r, t * 5 + u] = W11[t, d]
    l2a = np.ascontiguousarray(l2[0]).astype(BF)
    l2b = np.ascontiguousarray(l2[1]).astype(BF)
    m2s = M2.T.reshape(5, MC, O0).transpose(1, 0, 2)  # [120, 5, 3]
    m2p = np.zeros((MC, 15), np.float32)
    for m in range(5):
        m2p[:, 3 * m:3 * m + O0] = m2s[:, m, :]
    m2t = np.ascontiguousarray(m2p).astype(BF)

    if "nc" not in _CACHE:
        _CACHE["nc"] = _build_nc()
    nc = _CACHE["nc"]

    in_maps = []
    for c in range(NCORES):
        xs = x[c * BC : (c + 1) * BC]
        xtc = xs.transpose(1, 2, 0).reshape(KF, BC).astype(BF)
        in_maps.append({"xt": xtc, "ablk": ablk, "l2a": l2a, "l2b": l2b,
                        "m2t": m2t, "b1": b1v})

    res = run_bass_kernel_spmd(nc, in_maps, core_ids=list(range(NCORES)), trace=_trace)
    outs = [np.asarray(res.results[c]["outT"], np.float32)
                .reshape(4, 3, BC).sum(axis=0) for c in range(NCORES)]
    full = np.concatenate(outs, axis=1).T + b2v[None, :]
    if _trace:
        kernel.last_exec_time_ns = res.exec_time_ns
    return full.astype(np.float32)



# revision 23
# speedup vs baseline: 3.4314x; 1.0541x over previous
"""Trainium2 Bass kernel for nn_BL_36721970381090 (dense_mlp).

Math: the reference network
    item1 = einsum("td,bds->bts", W11, x)
    item2 = relu(einsum("bts,us->btu", item1, fc2_w) + bias1)
    item3 = einsum("ot,btu->bou", W12, item2)
    out   = (einsum("bou,pu->bop", item3, fc4_w) + bias2)[..., 0]
collapses (Kronecker identity) to a plain 2-layer MLP applied per batch row:
    out[b] = M2 @ relu(M1 @ vec(x[b]) + b1) + b2
with M1 = kron(W11, fc2_w) [600, 400], M2 = kron(W12, fc4_w) [3, 600],
b1 = bias1.reshape(600), b2 = bias2[:, 0].

Strategy: pure data parallel over 8 NeuronCores (batch split 131072 -> 8 x
16384). Host pre-transposes x to feature-major xT [400, Bc] per core and casts
to bf16 (input is the only large tensor; bf16 halves HBM traffic and doubles
PE throughput vs fp32's 2-pass matmul). On-chip: feature-major pipeline with
batch in the moving free dim - no on-chip transposes at all.
  layer1: psum[(t,u) chunk 120, b 512] += M1T_k[100,120].T @ xT_k[100,512]
          (4 K-chunks x 5 M-chunks)
  relu+bias1 on ScalarE (PSUM -> SBUF, cast to bf16)
  layer2: psum[3, b 512] += M2T_m[120,3].T @ relu_m[120,512]  (5 chunks)
  bias2 folded on host after gather.
"""

import numpy as np
import ml_dtypes
from contextlib import ExitStack

import concourse.bass as bass
import concourse.bacc as bacc
import concourse.mybir as mybir
from concourse.bass import ds
from concourse.tile import TileContext
from concourse.bass_utils import run_bass_kernel_spmd

B, D1, D2 = 131072, 40, 10
T0, T1, O0 = 120, 5, 3
NCORES = 8
BC = B // NCORES          # 16384 batch per core
KF = D1 * D2              # 400 input features (d, s)
MF = T0 * T1              # 600 hidden features (t, u)
KC = 100                  # K-chunk (4 chunks of 100 partitions)
MC = 120                  # M-chunk (5 chunks of 120 partitions)
NB = 512                  # matmul free-dim block (1 PSUM bank fp32)
NBD = 2048                # DMA block (4 x NB)

F32 = mybir.dt.float32
BF16 = mybir.dt.bfloat16
BF = ml_dtypes.bfloat16
RELU = mybir.ActivationFunctionType.Relu
COPY = mybir.ActivationFunctionType.Copy
ADD = mybir.AluOpType.add
MAX = mybir.AluOpType.max

_CACHE = {}


def _build_nc():
    nc = bacc.Bacc()
    xt = nc.dram_tensor("xt", (KF, BC), BF16, kind="ExternalInput")
    ablk = nc.dram_tensor("ablk", (KC, 256), BF16, kind="ExternalInput")
    l2a = nc.dram_tensor("l2a", (128, MF), BF16, kind="ExternalInput")
    l2b = nc.dram_tensor("l2b", (128, MF), BF16, kind="ExternalInput")
    m2t = nc.dram_tensor("m2t", (MC, 640), BF16, kind="ExternalInput")
    b1 = nc.dram_tensor("b1", (MC, 5), F32, kind="ExternalInput")
    outT = nc.dram_tensor("outT", (O0, BC), F32, kind="ExternalOutput")

    nm = MF // MC  # 5

    with TileContext(nc) as tc, ExitStack() as ctx:
        consts = ctx.enter_context(tc.tile_pool(name="consts", bufs=1))
        xpool = ctx.enter_context(tc.tile_pool(name="xp", bufs=3))
        # first batch-block x DMAs BEFORE the consts: they are the critical
        # path to the first matmul (trigger instrs serialize on Sync)
        x00 = [xpool.tile([KC, NB], BF16, tag=f"w{k}_0", name=f"x0_{k}_0")
               for k in range(4)]
        for k in range(4):
            nc.sync.dma_start(x00[k][:, :], xt[ds(k * KC, KC), ds(0, NB)])
        a_sb = consts.tile([KC, 256], BF16, tag="a")
        nc.sync.dma_start(a_sb[:, :], ablk[:, :])
        l2_sb = [consts.tile([128, MF], BF16, tag=f"l2_{p}", name=f"l2sb{p}") for p in range(2)]
        nc.sync.dma_start(l2_sb[0][:, :], l2a[:, :])
        nc.sync.dma_start(l2_sb[1][:, :], l2b[:, :])
        m2_sb = consts.tile([MC, 640], BF16, tag="m2")
        nc.sync.dma_start(m2_sb[:, :], m2t[:, :])
        b1_sb = consts.tile([MC, nm], F32, tag="b1")
        nc.sync.dma_start(b1_sb[:, :], b1[:, :])
        opool = ctx.enter_context(tc.tile_pool(name="op", bufs=2))
        zpool = ctx.enter_context(tc.tile_pool(name="zp", bufs=3))
        rpool = ctx.enter_context(tc.tile_pool(name="rp", bufs=6))
        pzp = ctx.enter_context(tc.tile_pool(name="pz", bufs=2, space="PSUM"))
        ps1p = ctx.enter_context(tc.tile_pool(name="ps1", bufs=4, space="PSUM"))
        ps2p = ctx.enter_context(tc.tile_pool(name="ps2", bufs=2, space="PSUM"))

        for blk in range(BC // NBD):
            if blk == 0:
                x0 = [x00] + [
                    [xpool.tile([KC, NB], BF16, tag=f"w{k}_{jj}", name=f"x0_{k}_{jj}")
                     for k in range(4)]
                    for jj in range(1, NBD // NB)
                ]
                for jj in range(1, NBD // NB):
                    for k in range(4):
                        nc.sync.dma_start(
                            x0[jj][k][:, :],
                            xt[ds(k * KC, KC), ds(jj * NB, NB)],
                        )
            else:
                xk = [xpool.tile([KC, NBD], BF16, tag=f"x{k}", name=f"xk{k}") for k in range(4)]
                for k in range(4):
                    nc.sync.dma_start(xk[k][:, :], xt[ds(k * KC, KC), ds(blk * NBD, NBD)])
            for jj in range(NBD // NB):
                def xs(k):
                    return (x0[jj][k][:, :] if blk == 0
                            else xk[k][:, ds(jj * NB, NB)])
                # stage 1: z pair tiles [128, 512]; chunk 2p -> rows 0:64,
                # chunk 2p+1 -> rows 64:128 (zero-padded cols 50-63 in ablk)
                ztiles = []
                for p in range(2):
                    pz = pzp.tile([128, NB], F32, tag="pz", name=f"pz{p}{jj}")
                    nc.tensor.matmul(pz[0:128, :], a_sb[:, 0:128], xs(2 * p),
                                     start=True, stop=False)
                    nc.tensor.matmul(pz[0:128, :], a_sb[:, 128:256], xs(2 * p + 1),
                                     start=False, stop=True)
                    z = zpool.tile([128, NB], BF16, tag=f"z{p}", name=f"zt{p}{jj}")
                    if p == 0:
                        nc.scalar.activation(z[:, :], pz[:, :], COPY)
                    else:
                        nc.vector.tensor_copy(z[:, :], pz[:, :])
                    ztiles.append(z)
                # stage 2 + relu + layer 2
                rtiles = []
                for m in range(nm):
                    pp = ps1p.tile([MC, NB], F32, tag="ps1", name=f"pp{m}{jj}")
                    for p in range(2):
                        nc.tensor.matmul(
                            pp[:, :], l2_sb[p][:, ds(m * MC, MC)], ztiles[p][:, :],
                            start=(p == 0), stop=(p == 1),
                        )
                    r = rpool.tile([MC, NB], BF16, tag=f"r{m}", name=f"rt{m}{jj}")
                    if m < 3:
                        nc.scalar.activation(r[:, :], pp[:, :], RELU,
                                             bias=b1_sb[:, ds(m, 1)])
                    else:
                        nc.vector.tensor_scalar(r[:, :], pp[:, :],
                                                b1_sb[:, ds(m, 1)], 0.0,
                                                op0=ADD, op1=MAX)
                    rtiles.append(r)
                if jj == 0:
                    osb = opool.tile([O0, NBD], F32, tag="osb")
                ps2 = ps2p.tile([128, NB], F32, tag="ps2", name=f"ps2{jj}")
                for m in range(nm):
                    nc.tensor.matmul(
                        ps2[:, :], m2_sb[:, ds(m * 128, 128)], rtiles[m][:, :],
                        start=(m == 0), stop=(m == nm - 1),
                    )
                nc.vector.tensor_copy(osb[:, ds(jj * NB, NB)], ps2[0:3, :])
            nc.sync.dma_start(outT[:, ds(blk * NBD, NBD)], osb[:, :])
    nc.finalize()
    return nc


def kernel(x, W11, fc2_w, bias1, W12, fc4_w, bias2, _trace=False):
    x = np.asarray(x, dtype=np.float32)
    W11 = np.asarray(W11, np.float32)
    fc2_w = np.asarray(fc2_w, np.float32)
    M2 = np.kron(np.asarray(W12, np.float32), np.asarray(fc4_w, np.float32))
    b1v = np.ascontiguousarray(np.asarray(bias1, np.float32).reshape(5, MC).T)
    b2v = np.asarray(bias2, np.float32)[:, 0]

    # stage-1 stationary: [(dl,s) 100, (u,dl') 50 + 14 zero pad]
    A50 = np.einsum("us,de->dsue", fc2_w,
                    np.eye(10, dtype=np.float32)).reshape(KC, 50)
    A = np.zeros((KC, 256), np.float32)
    A[:, 0:50] = A50
    A[:, 192:242] = A50
    ablk = np.ascontiguousarray(A).astype(BF)
    # stage-2 stationaries: pair p rows = chunk 2p (0:50), pad, chunk 2p+1 (64:114), pad
    l2 = np.zeros((2, 128, MF), np.float32)
    for p in range(2):
        for r in range(128):
            if r < 50:
                u, dl, d = r // 10, r % 10, 10 * (2 * p) + (r % 10)
            elif 64 <= r < 114:
                u, dl, d = (r - 64) // 10, (r - 64) % 10, 10 * (2 * p + 1) + ((r - 64) % 10)
            else:
                continue
            for t in range(T0):
                l2[p, r, t * 5 + u] = W11[t, d]
    l2a = np.ascontiguousarray(l2[0]).astype(BF)
    l2b = np.ascontiguousarray(l2[1]).astype(BF)
    m2s = M2.T.reshape(5, MC, O0).transpose(1, 0, 2)  # [120, 5, 3]
    m2p = np.zeros((MC, 640), np.float32)
    for m in range(5):
        m2p[:, 128 * m:128 * m + O0] = m2s[:, m, :]
    m2t = np.ascontiguousarray(m2p).astype(BF)

    if "nc" not in _CACHE:
        _CACHE["nc"] = _build_nc()
    nc = _CACHE["nc"]

    in_maps = []
    for c in range(NCORES):
        xs = x[c * BC : (c + 1) * BC]
        xtc = xs.transpose(1, 2, 0).reshape(KF, BC).astype(BF)
        in_maps.append({"xt": xtc, "ablk": ablk, "l2a": l2a, "l2b": l2b,
                        "m2t": m2t, "b1": b1v})

    res = run_bass_kernel_spmd(nc, in_maps, core_ids=list(range(NCORES)), trace=_trace)
    outs = [np.asarray(res.results[c]["outT"], np.float32) for c in range(NCORES)]
    full = np.concatenate(outs, axis=1).T + b2v[None, :]
    if _trace:
        kernel.last_exec_time_ns = res.exec_time_ns
    return full.astype(np.float32)

